# revision 11
# baseline (speedup 1.0000x reference)
"""Trainium2 Bass kernel for nn_LowRankSig_HigherOrder — v5.

Math (per example, T=2048, U=64, F=64 incl. time channel), with
summation-by-parts to eliminate the outer scans:
  dx[t] = Xa[t]-Xa[t-1]; ya[t] = Xa[t-1]-Xa[0]
  streams: yb = ya+dx/2, yc = ya/2+dx/6, yd2 = ya/3+dx/12,
           yr[t] = Xa[T-1]-Xa[t]
  projections (one matmul pass each unless noted):
    z1=yb@K1, m2=dx@K2, w2=yb@K3, w2b=yc@K3, m4=dx@K4,
    S5=yr@K5, m5=dx@K5, w3=yb@K6, w3b=yc@K6, w3c'=yd2@K6,
    m7=dx@K7, m8=dx@K8, m9h=dx@(K9/2), SA9=yr@K9+dx@(K9/2) (2-pass)
  out = d0@K0
      + Sum_t m2*z1                                   (T1)
      + Sum_t m4*(w2*S5 + w2b*m5)                     (T2)
      + Sum_t m8*(EB2*SA9 + qq)                       (T3)
        p = m7*w3; EB2 = ecum(p)  (the only scan)
        w = m7*w3b; v3c' = m7*w3c'
        qq = w*SA9 + (v3c'-w)*m9h   (computed DURING the scan)
  Identities: Sum_t A*ecum(B) = Sum_t B*(yr@K_A)  [telescoped tail sum];
  (EB2+w)*SA9 + (v3c'-w)*m9h = EB2*SA9 + qq.

Engine facts measured on this HW (v4.3 trace):
  matmul 512-col fp16 379ns med (+101ns ldweights); Act copy-half 1.11us;
  DVE STT-half 1.21us (always 1x); DVE TT-2048 bf16 1.21us (2x);
  DVE scan 4.42us; Pool TT-2048 4.1us (SBUF-only; PSUM is forbidden
  to GPSIMD); in-place DVE TT loses the 2x mode -> never alias.

Plan: PE run-ahead via a single 4-deep PSUM ring; DMA spread across 4
DGE queues in first-use order; pair-staggered emission trailing-first;
Pool takes only slack-window SBUF TTs (v3c', e2, q3, qq, f12, w);
DVE: scan halves (chained), p/f2 PSUM TTs, 3 STT-accum terms, 2x TTs.

Sharding: pure data parallel, 4 examples per core, packed 2-per-128-partitions.
"""

import numpy as np

import concourse.bass as bass
import concourse.bass_utils as _bu
import concourse.mybir as mybir
import concourse.tile as tile
from concourse.bass_utils import run_bass_kernel_spmd
from bass_rust import ScopedClock


def _patched_drain_and_barrier(self, tick_clock, wait_clock):
    """Split the final drain's sem waits across multiple drain instructions
    (this walrus build rejects >1-2 sync waits per instruction)."""
    drain_inst = self.nc.sync.drain()
    wait_clock.add_sem_waits(drain_inst.ins, ScopedClock({None: tick_clock.global_clock}))
    si = drain_inst.ins.sync_info
    if si is not None and si.on_wait and len(si.on_wait) > 1:
        waits = list(si.on_wait)
        ups = list(si.on_update or [])
        drain_inst.ins.sync_info = mybir.SyncInfo(on_wait=waits[:1], on_update=ups)
        for w in waits[1:]:
            d2 = self.nc.sync.drain()
            d2.ins.sync_info = mybir.SyncInfo(on_wait=[w], on_update=[])

    self.nc.all_engine_barrier()
    popped = self.nc._tile_sem_poison_stack.pop()
    assert popped is self._sem_poison
    self.nc.clear_and_free_semaphores(list(self.sems.allocated().values()))
    self.nc.all_engine_barrier()


tile.TileContext._drain_and_barrier = _patched_drain_and_barrier


def _sanitize_waits(nc, limit=1):
    """Move excess sem waits onto same-engine NOPs inserted just before."""
    import bass_rust

    counter = [0]
    for f in nc.m.functions:
        for blk in f.blocks:
            il = blk.instructions
            i = 0
            while i < len(il):
                inst = il[i]
                si = inst.sync_info
                waits = list(si.on_wait) if (si is not None and si.on_wait) else []
                if len(waits) > limit:
                    keep, extra = waits[:limit], waits[limit:]
                    inst.sync_info = mybir.SyncInfo(
                        on_wait=keep, on_update=list(si.on_update or [])
                    )
                    for j in range(0, len(extra), limit):
                        counter[0] += 1
                        nop = bass_rust.InstNoOp(
                            name=f"waitnop-{counter[0]}", ins=[], outs=[]
                        )
                        nop.engine = inst.engine
                        nop.sync_info = mybir.SyncInfo(
                            on_wait=extra[j : j + limit], on_update=[]
                        )
                        il.insert(i, nop)
                        i += 1
                i += 1
    return counter[0]


B, T, FX = 32, 2048, 63
U = 64
NCORES = 8
BL = B // NCORES
NPAIR = BL // 2

FP = mybir.dt.float32
FH = mybir.dt.float16
BF = mybir.dt.bfloat16
AluOp = mybir.AluOpType

(SK0, SK1, SK2, SK3, SK4, SK5, SK6, SK7, SK8, SK9, SK9H) = range(11)
NSLICE = 11

STREAMS = ["dx", "yb", "yc", "yd2", "yr", "dxT", "ybT"]


class PairState:
    pass


def build_nc(sanitize=True):
    nc = bass.Bass("TRN2", target_bir_lowering=False, debug=False)
    st_d = {
        s: nc.dram_tensor(f"{s}p", [NPAIR, 128, T], FH, kind="ExternalInput")
        for s in STREAMS
    }
    d0_d = nc.dram_tensor("d0p", [NPAIR, 128, 1], FH, kind="ExternalInput")
    kst_d = nc.dram_tensor("kst", [128, NSLICE * 128], FH, kind="ExternalInput")
    out_d = nc.dram_tensor("out", [NPAIR, 128, 1], FP, kind="ExternalOutput")

    with tile.TileContext(nc) as tc:
        with (
            tc.tile_pool(name="pool", bufs=2) as pool,
            tc.tile_pool(name="psum", bufs=4, space="PSUM") as psum,
            tc.tile_pool(name="kpool", bufs=1) as kpool,
        ):
            kst = kpool.tile([128, NSLICE * 128], FH, tag="kst", name="kst")
            nc.sync.dma_start(kst[:], kst_d.ap())
            zz = kpool.tile([128, T], BF, tag="zz", name="zz")
            nc.gpsimd.memset(zz[:], 0.0)
            ones = kpool.tile([128, 1], FH, tag="ones", name="ones")
            nc.gpsimd.memset(ones[:], 1.0)

            # DMA: spread streams across 4 DGE queues, in first-use order,
            # leading pair first.
            sts = []
            for p in range(NPAIR):
                st = PairState()
                st.p = p
                st.s = {}
                for s in STREAMS:
                    st.s[s] = pool.tile([128, T], FH, tag=s, name=f"{s}{p}")
                st.d0 = pool.tile([128, 1], FH, tag="d0", name=f"d0{p}")
                st.acc = kpool.tile([128, 12], FP, tag=f"acc{p}", name=f"acc{p}")
                sts.append(st)

            qeng = {"dx": nc.gpsimd, "yb": nc.sync, "yc": nc.sync,
                    "yd2": nc.gpsimd, "yr": nc.sync, "dxT": nc.gpsimd,
                    "ybT": nc.gpsimd}
            for s in ["dx", "yb", "yr", "yc", "yd2", "dxT", "ybT"]:
                for p in range(NPAIR):
                    for h in range(2):
                        sl = slice(h * 1024, (h + 1) * 1024)
                        qeng[s].dma_start(sts[p].s[s][:, sl], st_d[s][p, :, sl])
            for p in range(NPAIR):
                nc.sync.dma_start(sts[p].d0[:], d0_d[p])

            def proj(terms, name, p):
                """One projection pass into the shared PSUM ring:
                2 halves x 2 chunks of 512 cols."""
                halves = []
                for h in range(2):
                    ps = psum.tile([128, 1024], FP, tag="ps", name=f"{name}ps{h}_{p}")
                    for j in range(2):
                        lo = h * 1024 + j * 512
                        for ti, (si, src) in enumerate(terms):
                            nc.tensor.matmul(
                                ps[:, j * 512 : (j + 1) * 512],
                                kst[:, si * 128 : (si + 1) * 128],
                                src[:, lo : lo + 512],
                                start=(ti == 0),
                                stop=(ti == len(terms) - 1),
                            )
                    halves.append(ps)
                return halves

            def copy_out(halves, name, p, bufs=None):
                dst = pool.tile(
                    [128, T], BF, tag=f"c_{name}", name=f"{name}c{p}", bufs=bufs
                )
                for h in range(2):
                    sl = slice(h * 1024, (h + 1) * 1024)
                    nc.scalar.copy(out=dst[:, sl], in_=halves[h][:])
                return dst

            def term(st, zt, mh, c0):
                """acc[:, c0+h] += sum_t zt[:, half h] * mh[h] (PSUM)."""
                for h in range(2):
                    scr = pool.tile(
                        [128, 1024], BF, tag="scr", name=f"scr{st.p}_{c0}_{h}", bufs=2
                    )
                    nc.vector.scalar_tensor_tensor(
                        out=scr[:],
                        in0=zt[:, h * 1024 : (h + 1) * 1024],
                        scalar=1.0,
                        in1=mh[h][:],
                        op0=AluOp.mult,
                        op1=AluOp.mult,
                        accum_out=st.acc[:, c0 + h : c0 + h + 1],
                    )

            def tt_psum(out, sb, ph_halves, op=AluOp.mult):
                """out[128,T] = sb (SBUF) op proj-halves (PSUM), per half."""
                for h in range(2):
                    sl = slice(h * 1024, (h + 1) * 1024)
                    nc.vector.tensor_tensor(
                        out=out[:, sl], in0=sb[:, sl], in1=ph_halves[h][:], op=op
                    )

            def big(name, dt, p, bufs=None):
                return pool.tile([128, T], dt, tag=name, name=f"{name}{p}", bufs=bufs)

            # ---- phase functions (emitted pair-staggered below) ----
            def ph0(st):
                p = st.p
                m7h = proj([(SK7, st.s["dx"])], "m7", p)
                st.m7c = copy_out(m7h, "m7", p)
                w3h = proj([(SK6, st.s["yb"])], "w3", p)
                st.pp = big("pp", BF, p, bufs=1)
                tt_psum(st.pp, st.m7c, w3h)
                st.eb2 = big("eb2", BF, p)
                nc.gpsimd.memset(st.eb2[:, 0:1], 0.0)
                # split scan: h1 chains off h0's last element so post-scan
                # consumers of h0 can start while h1 still runs
                nc.vector.tensor_tensor_scan(
                    out=st.eb2[:, 1:1024], data0=zz[:, 0:1023],
                    data1=st.pp[:, 0:1023],
                    initial=0.0, op0=AluOp.add, op1=AluOp.add,
                )
                nc.vector.tensor_tensor_scan(
                    out=st.eb2[:, 1024:T], data0=zz[:, 1023 : T - 1],
                    data1=st.pp[:, 1023 : T - 1],
                    initial=st.eb2[:, 1023:1024], op0=AluOp.add, op1=AluOp.add,
                )
                s0 = psum.tile([128, 1024], FP, tag="ps", name=f"s0ps{p}")
                nc.tensor.matmul(
                    s0[:, 0:1],
                    kst[:, SK0 * 128 : (SK0 + 1) * 128],
                    st.d0[:, 0:1],
                    start=True, stop=True,
                )
                st.s0sb = pool.tile([128, 1], FP, tag="s0sb", name=f"s0sb{p}")
                nc.vector.tensor_copy(out=st.s0sb[:], in_=s0[:, 0:1])

            def ph1(st):
                # T1 = Sum_t m2*z1 entirely on PE via Gram matrices:
                #   G[g,f] = Sum_t yb[t,g] dx[t,f]  (16 chunk matmuls, PSUM acc)
                #   H = G.T@K1bd -> H[f,u] = Sum_t dx[t,f] z1[t,u]
                #   T1[u] = Sum_f K2bd[f,u]*H[f,u] = (K2bd .* H).T @ ones
                p = st.p
                gps = psum.tile([128, 1024], FP, tag="ps", name=f"gram{p}")
                for c in range(16):
                    cs = slice(c * 128, (c + 1) * 128)
                    nc.tensor.matmul(
                        gps[:, 0:128], st.s["ybT"][:, cs], st.s["dxT"][:, cs],
                        start=(c == 0), stop=(c == 15),
                    )
                gc = pool.tile([128, 128], FH, tag="gc", name=f"gc{p}")
                nc.scalar.copy(out=gc[:], in_=gps[:, 0:128])
                hps = psum.tile([128, 1024], FP, tag="ps", name=f"hps{p}")
                nc.tensor.matmul(
                    hps[:, 0:128], gc[:], kst[:, SK1 * 128 : (SK1 + 1) * 128],
                    start=True, stop=True,
                )
                hc = pool.tile([128, 128], FH, tag="hc", name=f"hc{p}")
                nc.scalar.copy(out=hc[:], in_=hps[:, 0:128])
                qt = pool.tile([128, 128], FH, tag="qt", name=f"qt{p}")
                nc.vector.tensor_tensor(
                    out=qt[:], in0=hc[:], in1=kst[:, SK2 * 128 : (SK2 + 1) * 128],
                    op=AluOp.mult,
                )
                t1ps = psum.tile([128, 1024], FP, tag="ps", name=f"t1ps{p}")
                nc.tensor.matmul(t1ps[:, 0:1], qt[:], ones[:], start=True, stop=True)
                st.t1sb = pool.tile([128, 1], FP, tag="t1sb", name=f"t1sb{p}")
                nc.vector.tensor_copy(out=st.t1sb[:], in_=t1ps[:, 0:1])

            def ph2(st):
                p = st.p
                w3bh = proj([(SK6, st.s["yc"])], "w3b", p)
                st.w3bc = copy_out(w3bh, "w3b", p)
                st.w = big("w", BF, p)
                nc.vector.tensor_tensor(
                    out=st.w[:], in0=st.m7c[:], in1=st.w3bc[:], op=AluOp.mult
                )
                w3ch = proj([(SK6, st.s["yd2"])], "w3cv", p)
                st.w3cc = copy_out(w3ch, "w3cv", p)
                st.v3c = big("v3c", BF, p, bufs=1)
                nc.gpsimd.tensor_tensor(
                    out=st.v3c[:], in0=st.m7c[:], in1=st.w3cc[:], op=AluOp.mult
                )
                sa9h = proj([(SK9, st.s["yr"]), (SK9H, st.s["dx"])], "sa9", p)
                st.sa9c = copy_out(sa9h, "sa9", p)
                m9hh = proj([(SK9H, st.s["dx"])], "m9h", p)
                st.m9hc = copy_out(m9hh, "m9h", p)

            def ph3(st):
                p = st.p
                st.q = big("q", BF, p, bufs=1)
                nc.vector.tensor_tensor(
                    out=st.q[:], in0=st.w[:], in1=st.sa9c[:], op=AluOp.mult
                )
                st.e2 = big("e2", BF, p, bufs=1)
                nc.vector.tensor_tensor(
                    out=st.e2[:], in0=st.v3c[:], in1=st.w[:], op=AluOp.subtract
                )
                st.q3 = big("q3", BF, p, bufs=1)
                nc.gpsimd.tensor_tensor(
                    out=st.q3[:], in0=st.e2[:], in1=st.m9hc[:], op=AluOp.mult
                )
                st.qq = big("qq", BF, p)
                nc.vector.tensor_tensor(
                    out=st.qq[:], in0=st.q[:], in1=st.q3[:], op=AluOp.add
                )

            def ph4(st):
                p = st.p
                st.g1 = big("g1", BF, p, bufs=1)
                nc.vector.tensor_tensor(
                    out=st.g1[:], in0=st.eb2[:], in1=st.sa9c[:], op=AluOp.mult
                )
                st.g = big("g", BF, p, bufs=1)
                nc.vector.tensor_tensor(
                    out=st.g[:], in0=st.g1[:], in1=st.qq[:], op=AluOp.add
                )
                m8h = proj([(SK8, st.s["dx"])], "m8", p)
                term(st, st.g, m8h, 2)

            def ph5(st):
                p = st.p
                w2h = proj([(SK3, st.s["yb"])], "w2", p)
                st.w2c = copy_out(w2h, "w2", p)
                s5h = proj([(SK5, st.s["yr"])], "s5", p)
                st.s5c = copy_out(s5h, "s5", p)
                st.f1 = big("f1", BF, p, bufs=1)
                nc.vector.tensor_tensor(
                    out=st.f1[:], in0=st.w2c[:], in1=st.s5c[:], op=AluOp.mult
                )
                m5h = proj([(SK5, st.s["dx"])], "m5", p)
                st.m5c = copy_out(m5h, "m5", p)
                w2bh = proj([(SK3, st.s["yc"])], "w2b", p)
                st.f2 = big("f2", BF, p, bufs=1)
                tt_psum(st.f2, st.m5c, w2bh)
                st.f12 = big("f12", BF, p, bufs=1)
                nc.gpsimd.tensor_tensor(
                    out=st.f12[:], in0=st.f1[:], in1=st.f2[:], op=AluOp.add
                )
                m4h = proj([(SK4, st.s["dx"])], "m4", p)
                term(st, st.f12, m4h, 4)

            def ph6(st):
                p = st.p
                red = pool.tile([128, 1], FP, tag="red", name=f"red{p}")
                nc.vector.tensor_reduce(
                    out=red[:], in_=st.acc[:, 2:6], axis=mybir.AxisListType.X,
                    op=AluOp.add,
                )
                red2 = pool.tile([128, 1], FP, tag="red2", name=f"red2{p}")
                nc.vector.tensor_tensor(
                    out=red2[:], in0=red[:], in1=st.t1sb[:], op=AluOp.add
                )
                outt = pool.tile([128, 1], FP, tag="outt", name=f"outt{p}")
                nc.vector.tensor_tensor(
                    out=outt[:], in0=red2[:], in1=st.s0sb[:], op=AluOp.add
                )
                nc.sync.dma_start(out_d[p], outt[:])

            # pair-staggered emission, trailing pair first: pair 1 trails
            # pair 0 by one phase and its (older, ready) ops are emitted
            # ahead of pair 0's next phase in every engine queue.
            phases = [ph0, ph5, ph2, ph3, ph1, ph4, ph6]
            sched = []
            for i in range(len(phases) + 1):
                if i >= 1:
                    sched.append((phases[i - 1], 1))
                if i < len(phases):
                    sched.append((phases[i], 0))
            for fn, p in sched:
                fn(sts[p])
    if sanitize:
        n = _sanitize_waits(nc)
        print(f"[kernel] split {n} excess sem waits onto NOPs")
    return nc


_CACHE = {}


def _get_nc():
    if "nc" not in _CACHE:
        _CACHE["nc"] = build_nc()
    return _CACHE["nc"]


def _bd(k64):
    b = np.zeros((128, 128), dtype=np.float32)
    b[:U, :U] = k64
    b[U:, U:] = k64
    return b


def _marshal(X, kernel):
    Xf = np.ascontiguousarray(X, dtype=np.float32)
    tch = np.arange(T, dtype=np.float32) * (2.0 / (T - 1.0)) - 1.0
    Xa = np.empty((B, T, U), dtype=np.float32)
    Xa[:, :, 0] = tch[None, :]
    Xa[:, :, 1:] = Xf

    ya = np.zeros_like(Xa)
    ya[:, 1:] = Xa[:, :-1] - Xa[:, :1]
    dxa = np.zeros_like(Xa)
    dxa[:, 1:] = Xa[:, 1:] - Xa[:, :-1]
    d0 = Xa[:, -1] - Xa[:, 0]  # [B, U]

    streams = {
        "dx": dxa,
        "yb": ya + 0.5 * dxa,
        "yc": 0.5 * ya + dxa / 6.0,
        "yd2": ya / 3.0 + dxa / 12.0,
        "yr": Xa[:, -1:, :] - Xa,
    }

    def pack(A):  # [B, T, U] -> [NCORES, NPAIR, 128, T]
        return np.ascontiguousarray(
            A.reshape(NCORES, NPAIR, 2, T, U).transpose(0, 1, 2, 4, 3)
        ).reshape(NCORES, NPAIR, 128, T)

    def packT(A):  # [B, T, U] -> [NCORES, NPAIR, 128tw, 16ck*(2e*64f)]
        return np.ascontiguousarray(
            A.reshape(NCORES, NPAIR, 2, 16, 128, U).transpose(0, 1, 4, 3, 2, 5)
        ).reshape(NCORES, NPAIR, 128, T)

    packed = {s: pack(a).astype(np.float16) for s, a in streams.items()}
    packed["dxT"] = packT(streams["dx"]).astype(np.float16)
    packed["ybT"] = packT(streams["yb"]).astype(np.float16)
    d0p = np.ascontiguousarray(d0.reshape(NCORES, NPAIR, 128, 1)).astype(np.float16)

    kf = np.asarray(kernel, dtype=np.float32)
    K = [kf[:, i, :] for i in range(10)]
    slices = [K[0], K[1], K[2], K[3], K[4], K[5], K[6], K[7], K[8], K[9], 0.5 * K[9]]
    kst16 = np.ascontiguousarray(
        np.stack([_bd(s) for s in slices]).transpose(1, 0, 2).reshape(128, NSLICE * 128)
    ).astype(np.float16)
    return packed, d0p, kst16


def run(X, kernel, trace=False):
    nc = _get_nc()
    packed, d0p, kst = _marshal(X, kernel)
    in_maps = [
        {
            **{f"{s}p": packed[s][c] for s in STREAMS},
            "d0p": d0p[c],
            "kst": kst,
        }
        for c in range(NCORES)
    ]
    res = run_bass_kernel_spmd(nc, in_maps, list(range(NCORES)), trace=trace)
    out = np.stack([r["out"] for r in res.results])
    out = out.reshape(NCORES, NPAIR, 2, U).reshape(B, U)
    return out, res


def kernel(X, kernel):
    out, _ = run(X, kernel)
    return out
